# revision 14
# baseline (speedup 1.0000x reference)
"""CAM-module (channel attention, H=W=1) Trainium2 kernel.

Reference math (B=32, C=2048):
    x    = concat([x1, x2], 1) @ W.T + b                  # (B, C)
    E    = x[:, :, None] * x[:, None, :]                  # (B, C, C)
    A    = softmax(max_j(E) - E, axis=-1)                 # (B, C, C)
    out1 = A @ x1 + x1 ; out2 = A @ x2 + x2               # (B, C)

Key identities used here:
  * softmax(max_j(E) - E)[i, j] == softmax(-x_i * x_j)[j]  (row constants
    cancel in softmax).  With x ~ N(0,1), |x_i*x_j| < ~30, so exp(-x_i*x_j)
    is safely inside the f32 range and NO max-subtraction is needed:
        A[i, j] = exp(-x_i*x_j) / s_i,   s_i = sum_j exp(-x_i*x_j)
    Each 128-row attention tile is ONE ScalarE activation:
        e = Exp(xb * scale_p),  scale_p = -x_i  (per-partition scalar),
    with s_i accumulated for free via accum_out.
  * e is symmetric, so the matvec A @ x1 contracts over the PARTITION dim:
        num1[f] = sum_i x1[i] * e[i, f]   (PE matmul, accumulated over
    row-tiles), and out1[f] = num1[f] / s_f + x1[f].

Sharding: data-parallel over batch, 4 batches per core on 8 cores; the
(2048, 4096) weight is replicated.  Each core computes x for its batches
(PE matmul over W.T tiles), then streams 64 x 1 MiB attention tiles.
"""

import numpy as np

import concourse.bacc as bacc
import concourse.bass as bass
import concourse.mybir as mybir
import concourse.tile as tile
from concourse.masks import make_identity

F32 = mybir.dt.float32

B, C = 32, 2048
K2 = 2 * C  # 4096 contraction dim of the conv1x1
NCORES = 8
NB = B // NCORES  # batches per core = 4
RT = C // 128  # row tiles per batch = 16
KC = K2 // 128  # k chunks = 32

_prog_cache = {}
LAST_RESULTS = None  # BassKernelResults of the most recent run (for test.py)


def _build_program():
    nc = bacc.Bacc(None, target_bir_lowering=False)

    xcatT_d = nc.dram_tensor("xcatT3", [128, KC, NB], F32, kind="ExternalInput")
    wt_d = nc.dram_tensor("wt", [K2, C], F32, kind="ExternalInput")
    x12_d = nc.dram_tensor("x12col", [128, NB, RT, 2], F32, kind="ExternalInput")
    bcol_d = nc.dram_tensor("bcol", [128, NB, RT], F32, kind="ExternalInput")
    att_d = nc.dram_tensor("att", [NB, C, C], F32, kind="ExternalOutput")
    o1_d = nc.dram_tensor("o1c", [128, NB, RT], F32, kind="ExternalOutput")
    o2_d = nc.dram_tensor("o2c", [128, NB, RT], F32, kind="ExternalOutput")

    with tile.TileContext(nc) as tc:
        with (
            tc.tile_pool(name="consts", bufs=1) as consts,
            tc.tile_pool(name="stats", bufs=1) as stats,
            tc.tile_pool(name="small", bufs=2) as small,
            tc.tile_pool(name="wt", bufs=3) as wt_pool,
            tc.tile_pool(name="xb", bufs=2) as xb_pool,
            tc.tile_pool(name="e", bufs=6) as e_pool,
            tc.tile_pool(name="dram", bufs=1, space="DRAM") as dram_pool,
            tc.tile_pool(name="numacc", bufs=2) as numacc_pool,
            tc.tile_pool(name="psum_x", bufs=2, space="PSUM") as psum_x,
            tc.tile_pool(name="psum_t", bufs=1, space="PSUM") as psum_t,
            tc.tile_pool(name="psum_n", bufs=2, space="PSUM") as psum_n,
            tc.tile_pool(name="psum_d", bufs=1, space="PSUM") as psum_d,
        ):

            d_dummy = psum_d.tile([1, 1], F32, tag="dummy")

            def pe_touch(col_ap):
                """Dummy 1x1 matmul reading `col_ap` ([K, 1] slice).  PE
                matmuls can encode only ONE semaphore wait (S3_LW), so each
                real matmul must have at most one unobserved dependency;
                this advances the PE's vector clock past `col_ap`'s producer
                before the real matmuls are issued.  All touches share one
                persistent PSUM scratch (same-engine WAW needs no sem).
                """
                nc.tensor.matmul(d_dummy[:], col_ap, col_ap, start=True, stop=True)

            ident = consts.tile([128, 128], F32)
            make_identity(nc, ident[:])

            xcatT_sb = consts.tile([128, KC, NB], F32)
            nc.gpsimd.dma_start(out=xcatT_sb[:], in_=xcatT_d[:])
            x12_sb = consts.tile([128, NB, RT, 2], F32)
            nc.gpsimd.dma_start(out=x12_sb[:], in_=x12_d[:])
            bcol_sb = consts.tile([128, NB, RT], F32)
            nc.gpsimd.dma_start(out=bcol_sb[:], in_=bcol_d[:])

            pe_touch(xcatT_sb[:, 0, 0:1])
            pe_touch(x12_sb[:, 0, 0, 0:1])

            # ---- Phase 0: x = concat(x1,x2) @ W.T + b, column layout ----
            # Per k-chunk the PE writes 16 single-matmul groups into one PSUM
            # bank (start=True only clears has_written bits, data of other
            # slices stays); the k-reduction accumulates on the DVE in SBUF.
            xacc = stats.tile([128, NB, RT], F32)
            for kc in range(KC):
                wt_t = wt_pool.tile([128, C], F32, tag="wt")
                nc.sync.dma_start(out=wt_t[:], in_=wt_d[kc * 128 : (kc + 1) * 128, :])
                pe_touch(wt_t[:, 0:1])
                pk = psum_x.tile([128, NB, RT], F32, tag="pk")
                for ic in range(RT):
                    nc.tensor.matmul(
                        pk[:, :, ic],
                        wt_t[:, ic * 128 : (ic + 1) * 128],
                        xcatT_sb[:, kc, :],
                        start=True,
                        stop=True,
                    )
                if kc == 0:
                    nc.vector.tensor_copy(xacc[:], pk[:])
                else:
                    nc.vector.tensor_add(xacc[:], xacc[:], pk[:])

            xcol_sb = stats.tile([128, NB, RT], F32)
            nc.vector.tensor_add(xcol_sb[:], xacc[:], bcol_sb[:])
            scol = stats.tile([128, NB, RT], F32)
            nc.vector.tensor_scalar_mul(scol[:], xcol_sb[:], -1.0)

            # Row layout x via PE transpose: out[q=b*16+rt, p] = x[b, rt*128+p]
            xrowT_p = psum_t.tile([NB * RT, 128], F32, tag="t")
            nc.tensor.transpose(
                xrowT_p[:], xcol_sb[:].rearrange("p b r -> p (b r)"), ident[:]
            )
            xrow_sb = stats.tile([NB * RT, 128], F32)
            nc.vector.tensor_copy(xrow_sb[:], xrowT_p[:])
            # Spill rows to DRAM; flat layout == x[b] rows contiguous.
            xs_d = dram_pool.tile([NB * RT, 128], F32)
            nc.sync.dma_start(out=xs_d[:], in_=xrow_sb[:])

            s_col = stats.tile([128, NB, RT], F32)
            rcp_col = stats.tile([128, NB, RT], F32)
            o1_sb = stats.tile([128, NB, RT], F32)
            o2_sb = stats.tile([128, NB, RT], F32)

            xs_base = xs_d[:]

            # ---- Phase 1: attention tiles + out1/out2 numerators ----
            for b in range(NB):
                # broadcast x[b, :] to all 128 partitions
                xb_sb = xb_pool.tile([128, C], F32, tag="xb")
                bcast = bass.AP(
                    tensor=xs_base.tensor,
                    offset=xs_base.offset + b * C,
                    ap=[[0, 128], [1, C]],
                )
                nc.gpsimd.dma_start(out=xb_sb[:], in_=bcast)

                # num_acc[p, q, m] = sum_i xm[i] * exp(-x_i * x_{q*128+p});
                # by symmetry of exp(-x_i x_j) this is the out_m numerator.
                num_acc = numacc_pool.tile([128, RT, 2], F32, tag="numacc")
                for rt in range(RT):
                    e_t = e_pool.tile([128, C], F32, tag="e")
                    nc.scalar.activation(
                        e_t[:],
                        xb_sb[:],
                        mybir.ActivationFunctionType.Exp,
                        bias=0.0,
                        scale=scol[:, b, rt : rt + 1],
                        accum_out=s_col[:, b, rt : rt + 1],
                    )
                    # e as stationary (f32): 16 single-matmul groups into one
                    # PSUM bank, then DVE-accumulate over rt in SBUF.
                    pe_touch(e_t[:, 0:1])
                    nrt = psum_n.tile([128, RT, 2], F32, tag="nrt")
                    for q in range(RT):
                        nc.tensor.matmul(
                            nrt[:, q, :],
                            e_t[:, q * 128 : (q + 1) * 128],
                            x12_sb[:, b, rt, :],
                            start=True,
                            stop=True,
                        )
                    if rt == 0:
                        nc.vector.tensor_copy(num_acc[:], nrt[:])
                    else:
                        nc.vector.tensor_add(num_acc[:], num_acc[:], nrt[:])
                    nc.vector.reciprocal(
                        rcp_col[:, b, rt : rt + 1], s_col[:, b, rt : rt + 1]
                    )
                    nc.vector.tensor_scalar_mul(
                        e_t[:], e_t[:], rcp_col[:, b, rt : rt + 1]
                    )
                    nc.sync.dma_start(
                        out=att_d[b, rt * 128 : (rt + 1) * 128, :], in_=e_t[:]
                    )

                # finals: out_m[f] = num_m[f] / s_f + xm[f], in column layout
                t1 = small.tile([128, RT], F32, tag="t1")
                nc.vector.tensor_mul(t1[:], num_acc[:, :, 0], rcp_col[:, b, :])
                nc.vector.tensor_add(o1_sb[:, b, :], t1[:], x12_sb[:, b, :, 0])
                t2 = small.tile([128, RT], F32, tag="t2")
                nc.vector.tensor_mul(t2[:], num_acc[:, :, 1], rcp_col[:, b, :])
                nc.vector.tensor_add(o2_sb[:, b, :], t2[:], x12_sb[:, b, :, 1])

            nc.sync.dma_start(out=o1_d[:], in_=o1_sb[:])
            nc.sync.dma_start(out=o2_d[:], in_=o2_sb[:])

    nc.finalize()  # Bacc: runs compile() (wait legalization etc.) + freeze
    return nc


def get_program():
    if "nc" not in _prog_cache:
        _prog_cache["nc"] = _build_program()
    return _prog_cache["nc"]


def make_in_maps(x1, x2, W, b):
    """Host-side shard + relayout. Pure data movement, no math beyond what
    the reference's concatenate does."""
    x1 = np.asarray(x1, np.float32)
    x2 = np.asarray(x2, np.float32)
    W = np.asarray(W, np.float32)
    b = np.asarray(b, np.float32)

    wt = np.ascontiguousarray(W.T)  # (4096, 2048)
    # bcol[p, b, rt] = bias[rt*128 + p]
    bcol = np.broadcast_to(
        np.ascontiguousarray(b.reshape(RT, 128).T)[:, None, :], (128, NB, RT)
    )
    bcol = np.ascontiguousarray(bcol)

    in_maps = []
    for c in range(NCORES):
        bs = slice(c * NB, (c + 1) * NB)
        xcat = np.concatenate([x1[bs], x2[bs]], axis=1)  # (4, 4096)
        # xcatT3[p, kc, b] = xcat[b, kc*128 + p]
        xcatT3 = np.ascontiguousarray(xcat.T.reshape(KC, 128, NB).transpose(1, 0, 2))
        # x12col[p, b, rt, m] = xm[bs][b, rt*128 + p]
        x1c = x1[bs].reshape(NB, RT, 128).transpose(2, 0, 1)
        x2c = x2[bs].reshape(NB, RT, 128).transpose(2, 0, 1)
        x12 = np.ascontiguousarray(np.stack([x1c, x2c], axis=-1))
        in_maps.append(
            {"xcatT3": xcatT3, "wt": wt, "x12col": x12, "bcol": bcol}
        )
    return in_maps


def assemble_outputs(results):
    att = np.concatenate([r["att"] for r in results], axis=0)  # (32, 2048, 2048)
    outs = []
    for key in ("o1c", "o2c"):
        # o[p, b, rt] -> out[b, rt*128+p]
        per_core = [
            r[key].transpose(1, 2, 0).reshape(NB, C) for r in results
        ]
        outs.append(np.concatenate(per_core, axis=0).astype(np.float32))
    return outs[0], outs[1], att.astype(np.float32, copy=False)


def kernel(x1, x2, W, b, _trace=False):
    global LAST_RESULTS
    from concourse.bass_utils import run_bass_kernel_spmd

    nc = get_program()
    in_maps = make_in_maps(x1, x2, W, b)
    res = run_bass_kernel_spmd(
        nc, in_maps, core_ids=list(range(NCORES)), trace=_trace
    )
    LAST_RESULTS = res
    return assemble_outputs(res.results)


# revision 19
# speedup vs baseline: 1.4738x; 1.4738x over previous
"""CAM-module (channel attention, H=W=1) Trainium2 kernel.

Reference math (B=32, C=2048):
    x    = concat([x1, x2], 1) @ W.T + b                  # (B, C)
    E    = x[:, :, None] * x[:, None, :]                  # (B, C, C)
    A    = softmax(max_j(E) - E, axis=-1)                 # (B, C, C)
    out1 = A @ x1 + x1 ; out2 = A @ x2 + x2               # (B, C)

Key identities used here:
  * softmax(max_j(E) - E)[i, j] == softmax(-x_i * x_j)[j]  (row constants
    cancel in softmax).  With x ~ N(0,1), |x_i*x_j| < ~30, so exp(-x_i*x_j)
    is safely inside the f32 range and NO max-subtraction is needed:
        A[i, j] = exp(-x_i*x_j) / s_i,   s_i = sum_j exp(-x_i*x_j)
    Each 128-row attention tile is ONE ScalarE activation:
        e = Exp(xb * scale_p),  scale_p = -x_i  (per-partition scalar),
    with s_i accumulated for free via accum_out.
  * exp(-x_i x_j) is symmetric, so the matvec A @ x1 can contract over
    either index:  num1[f] = sum_i x1[i] * e[i, f]  (PE matmul over the
    same tiles), and out1[f] = num1[f] / s_f + x1[f].

Sharding: the conv1x1 x = concat @ W.T is sharded over OUTPUT CHANNELS
(each core reads only W.T[:, c*256:(c+1)*256], 4 MiB) and computed for
all 32 batches; an AllToAll then hands each core the full-channel x rows
of ITS 4 batches (rank-independent access patterns).  The (B,C,C)
attention is batch-sharded: 4 batches per core, 64 MiB of tile writes.
"""

import numpy as np

import concourse.bacc as bacc
import concourse.bass as bass
import concourse.mybir as mybir
import concourse.tile as tile
from concourse.masks import make_identity

F32 = mybir.dt.float32
F32R = mybir.dt.float32r

B, C = 32, 2048
K2 = 2 * C  # 4096 contraction dim of the conv1x1
NCORES = 8
NB = B // NCORES  # batches per core = 4
RT = C // 128  # row tiles per batch = 16
KC = K2 // 128  # k chunks = 32
CS = C // NCORES  # output-channel slice per core = 256

_prog_cache = {}
LAST_RESULTS = None  # BassKernelResults of the most recent run (for test.py)


def _build_program():
    nc = bacc.Bacc(None, target_bir_lowering=False, num_devices=NCORES)

    xcatT_d = nc.dram_tensor("xcatT3", [128, KC, B], F32, kind="ExternalInput")
    wtsl_d = nc.dram_tensor("wtsl", [K2, CS], F32, kind="ExternalInput")
    x12_d = nc.dram_tensor("x12col", [128, NB, RT, 2], F32, kind="ExternalInput")
    b32_d = nc.dram_tensor("b32", [B, CS], F32, kind="ExternalInput")
    att_d = nc.dram_tensor("att", [NB, C, C], F32, kind="ExternalOutput")
    o1_d = nc.dram_tensor("o1c", [128, NB, RT], F32, kind="ExternalOutput")
    o2_d = nc.dram_tensor("o2c", [128, NB, RT], F32, kind="ExternalOutput")

    send_d = nc.dram_tensor("x_send", [B, CS], F32)
    recv_d = nc.dram_tensor("x_recv", [NCORES, NB, CS], F32)

    with tile.TileContext(nc) as tc:
        with (
            tc.tile_pool(name="consts", bufs=1) as consts,
            tc.tile_pool(name="stats", bufs=1) as stats,
            tc.tile_pool(name="small", bufs=2) as small,
            tc.tile_pool(name="xb", bufs=2) as xb_pool,
            tc.tile_pool(name="e", bufs=5) as e_pool,
            tc.tile_pool(name="er", bufs=3) as er_pool,
            tc.tile_pool(name="psum_x", bufs=1, space="PSUM") as psum_x,
            tc.tile_pool(name="psum_t", bufs=2, space="PSUM") as psum_t,
            tc.tile_pool(name="psum_num", bufs=1, space="PSUM") as psum_num,
        ):
            ident = consts.tile([128, 128], F32)
            make_identity(nc, ident[:])

            xcatT_sb = consts.tile([128, KC, B], F32)
            nc.gpsimd.dma_start(out=xcatT_sb[:], in_=xcatT_d[:])
            x12_sb = consts.tile([128, NB, RT, 2], F32)
            nc.gpsimd.dma_start(out=x12_sb[:], in_=x12_d[:])
            x12r_sb = consts.tile([128, NB, RT, 2], F32R)
            nc.gpsimd.tensor_copy(x12r_sb[:], x12_sb[:])
            b32_sb = consts.tile([B, CS], F32)
            nc.gpsimd.dma_start(out=b32_sb[:], in_=b32_d[:])
            wt3 = consts.tile([128, KC, CS], F32)
            nc.sync.dma_start(
                out=wt3[:], in_=wtsl_d[:].rearrange("(kc p) r -> p kc r", p=128)
            )

            # ---- Phase 0: x[:, my 256 channels] for ALL 32 batches ----
            xps = psum_x.tile([B, CS], F32)
            for kc in range(KC):
                nc.tensor.matmul(
                    xps[:],
                    xcatT_sb[:, kc, :],
                    wt3[:, kc, :],
                    start=(kc == 0),
                    stop=(kc == KC - 1),
                )
            xp_sb = stats.tile([B, CS], F32)
            nc.vector.tensor_add(xp_sb[:], xps[:], b32_sb[:])
            nc.sync.dma_start(out=send_d[:], in_=xp_sb[:])

            # AllToAll chunk g of send = batches [4g, 4g+4); each core
            # receives its own 4 batches x all 8 channel slices.
            nc.gpsimd.collective_compute(
                "AllToAll",
                mybir.AluOpType.bypass,
                replica_groups=[list(range(NCORES))],
                ins=[send_d[:]],
                outs=[recv_d[:]],
            )

            # scol[p, rt%2, b, rt//2] = -x[b, rt*128+p]
            recv_base = recv_d[:]
            scol = stats.tile([128, 2, NB, NCORES], F32)
            for par in range(2):
                for bb in range(NB):
                    nc.gpsimd.dma_start(
                        out=scol[:, par, bb],
                        in_=bass.AP(
                            tensor=recv_base.tensor,
                            offset=recv_base.offset + par * 128 + bb * CS,
                            ap=[[1, 128], [NB * CS, NCORES]],
                        ),
                    )
            nc.vector.tensor_scalar_mul(scol[:], scol[:], -1.0)

            s_col = stats.tile([128, NB, RT], F32)
            rcp_col = stats.tile([128, NB, RT], F32)
            o1_sb = stats.tile([128, NB, RT], F32)
            o2_sb = stats.tile([128, NB, RT], F32)

            # ---- Phase 1: attention tiles + out1/out2 numerators ----
            for b in range(NB):
                # broadcast x[b, :] (all channels) to all 128 partitions
                xb_sb = xb_pool.tile([128, C], F32, tag="xb")
                nc.gpsimd.dma_start(
                    out=xb_sb[:],
                    in_=bass.AP(
                        tensor=recv_base.tensor,
                        offset=recv_base.offset + b * CS,
                        ap=[[0, 128], [NB * CS, NCORES], [1, CS]],
                    ),
                )

                num_p = psum_num.tile([2, C], F32, tag="num")
                for rt in range(RT):
                    e_t = e_pool.tile([128, C], F32, tag="e")
                    nc.scalar.activation(
                        e_t[:],
                        xb_sb[:],
                        mybir.ActivationFunctionType.Exp,
                        bias=0.0,
                        scale=scol[:, rt % 2, b, rt // 2 : rt // 2 + 1],
                        accum_out=s_col[:, b, rt : rt + 1],
                    )
                    # tf32-rounded copy for the PE (f32r streams 4x faster);
                    # alternate engines to balance load.
                    e_r = er_pool.tile([128, C], F32R, tag="er")
                    if rt % 2 == 0:
                        nc.gpsimd.tensor_copy(e_r[:], e_t[:])
                    else:
                        nc.vector.tensor_copy(e_r[:], e_t[:])
                    for jc in range(4):
                        nc.tensor.matmul(
                            num_p[:, jc * 512 : (jc + 1) * 512],
                            x12r_sb[:, b, rt, :],
                            e_r[:, jc * 512 : (jc + 1) * 512],
                            start=(rt == 0),
                            stop=(rt == RT - 1),
                        )
                    nc.vector.reciprocal(
                        rcp_col[:, b, rt : rt + 1], s_col[:, b, rt : rt + 1]
                    )
                    nc.vector.tensor_scalar_mul(
                        e_t[:], e_t[:], rcp_col[:, b, rt : rt + 1]
                    )
                    nc.sync.dma_start(
                        out=att_d[b, rt * 128 : (rt + 1) * 128, :], in_=e_t[:]
                    )

                # finals: out_m[f] = num_m[f] / s_f + xm[f], in column layout
                num_sb = small.tile([2, C], F32, tag="numsb")
                nc.vector.tensor_copy(num_sb[:], num_p[:])
                numT_p = psum_t.tile([128, RT, 2], F32, tag="t")
                for rt in range(RT):
                    nc.tensor.transpose(
                        numT_p[:, rt, :],
                        num_sb[:, rt * 128 : (rt + 1) * 128],
                        ident[0:2, 0:2],
                    )
                t1 = small.tile([128, RT], F32, tag="t1")
                nc.vector.tensor_mul(t1[:], numT_p[:, :, 0], rcp_col[:, b, :])
                nc.vector.tensor_add(o1_sb[:, b, :], t1[:], x12_sb[:, b, :, 0])
                t2 = small.tile([128, RT], F32, tag="t2")
                nc.vector.tensor_mul(t2[:], numT_p[:, :, 1], rcp_col[:, b, :])
                nc.vector.tensor_add(o2_sb[:, b, :], t2[:], x12_sb[:, b, :, 1])

            nc.sync.dma_start(out=o1_d[:], in_=o1_sb[:])
            nc.sync.dma_start(out=o2_d[:], in_=o2_sb[:])

    nc.finalize()  # Bacc: runs compile() (wait legalization etc.) + freeze
    return nc


def get_program():
    if "nc" not in _prog_cache:
        _prog_cache["nc"] = _build_program()
    return _prog_cache["nc"]


def make_in_maps(x1, x2, W, b):
    """Host-side shard + relayout. Pure data movement, no math beyond what
    the reference's concatenate does."""
    x1 = np.asarray(x1, np.float32)
    x2 = np.asarray(x2, np.float32)
    W = np.asarray(W, np.float32)
    b = np.asarray(b, np.float32)

    wt = W.T  # (4096, 2048) view
    xcat = np.concatenate([x1, x2], axis=1)  # (32, 4096)
    # xcatT3[p, kc, b] = xcat[b, kc*128 + p], all batches, replicated
    xcatT3 = np.ascontiguousarray(xcat.T.reshape(KC, 128, B).transpose(1, 0, 2))

    in_maps = []
    for c in range(NCORES):
        bs = slice(c * NB, (c + 1) * NB)
        cs = slice(c * CS, (c + 1) * CS)
        wtsl = np.ascontiguousarray(wt[:, cs])  # (4096, 256)
        b32 = np.ascontiguousarray(np.broadcast_to(b[cs], (B, CS)))
        # x12col[p, b, rt, m] = xm[bs][b, rt*128 + p]
        x1c = x1[bs].reshape(NB, RT, 128).transpose(2, 0, 1)
        x2c = x2[bs].reshape(NB, RT, 128).transpose(2, 0, 1)
        x12 = np.ascontiguousarray(np.stack([x1c, x2c], axis=-1))
        in_maps.append(
            {"xcatT3": xcatT3, "wtsl": wtsl, "x12col": x12, "b32": b32}
        )
    return in_maps


def assemble_outputs(results):
    att = np.concatenate([r["att"] for r in results], axis=0)  # (32, 2048, 2048)
    outs = []
    for key in ("o1c", "o2c"):
        # o[p, b, rt] -> out[b, rt*128+p]
        per_core = [
            r[key].transpose(1, 2, 0).reshape(NB, C) for r in results
        ]
        outs.append(np.concatenate(per_core, axis=0).astype(np.float32))
    return outs[0], outs[1], att.astype(np.float32, copy=False)


def kernel(x1, x2, W, b, _trace=False):
    global LAST_RESULTS
    from concourse.bass_utils import run_bass_kernel_spmd

    nc = get_program()
    in_maps = make_in_maps(x1, x2, W, b)
    res = run_bass_kernel_spmd(
        nc, in_maps, core_ids=list(range(NCORES)), trace=_trace
    )
    LAST_RESULTS = res
    return assemble_outputs(res.results)


# revision 21
# speedup vs baseline: 2.1013x; 1.4258x over previous
"""CAM-module (channel attention, H=W=1) Trainium2 kernel.

Reference math (B=32, C=2048):
    x    = concat([x1, x2], 1) @ W.T + b                  # (B, C)
    E    = x[:, :, None] * x[:, None, :]                  # (B, C, C)
    A    = softmax(max_j(E) - E, axis=-1)                 # (B, C, C)
    out1 = A @ x1 + x1 ; out2 = A @ x2 + x2               # (B, C)

Key identities used here:
  * softmax(max_j(E) - E)[i, j] == softmax(-x_i * x_j)[j]  (row constants
    cancel in softmax).  With x ~ N(0,1), |x_i*x_j| < ~30, so exp(-x_i*x_j)
    is safely inside the f32 range and NO max-subtraction is needed:
        A[i, j] = exp(-x_i*x_j) / s_i,   s_i = sum_j exp(-x_i*x_j)
    Each 128-row attention tile is ONE ScalarE activation:
        e = Exp(xb * scale_p),  scale_p = -x_i  (per-partition scalar),
    with s_i accumulated for free via accum_out.
  * exp(-x_i x_j) is symmetric, so the matvec A @ x1 can contract over
    either index:  num1[f] = sum_i x1[i] * e[i, f]  (PE matmul over the
    same tiles), and out1[f] = num1[f] / s_f + x1[f].

Sharding: the conv1x1 x = concat @ W.T is sharded over OUTPUT CHANNELS
(each core reads only W.T[:, c*256:(c+1)*256], 4 MiB) and computed for
all 32 batches; an AllToAll then hands each core the full-channel x rows
of ITS 4 batches (rank-independent access patterns).  The (B,C,C)
attention is batch-sharded: 4 batches per core, 64 MiB of tile writes.
"""

import numpy as np

import concourse.bacc as bacc
import concourse.bass as bass
import concourse.mybir as mybir
import concourse.tile as tile
from concourse.masks import make_identity

F32 = mybir.dt.float32
F32R = mybir.dt.float32r

B, C = 32, 2048
K2 = 2 * C  # 4096 contraction dim of the conv1x1
NCORES = 8
NB = B // NCORES  # batches per core = 4
RT = C // 128  # row tiles per batch = 16
KC = K2 // 128  # k chunks = 32
CS = C // NCORES  # output-channel slice per core = 256

_prog_cache = {}
LAST_RESULTS = None  # BassKernelResults of the most recent run (for test.py)


def _build_program():
    nc = bacc.Bacc(None, target_bir_lowering=False, num_devices=NCORES)

    xcatT_d = nc.dram_tensor("xcatT3", [128, KC, B], F32, kind="ExternalInput")
    wtsl_d = nc.dram_tensor("wtsl", [K2, CS], F32, kind="ExternalInput")
    x12_d = nc.dram_tensor("x12col", [128, NB, RT, 2], F32, kind="ExternalInput")
    b32_d = nc.dram_tensor("b32", [B, CS], F32, kind="ExternalInput")
    att_d = nc.dram_tensor("att", [NB, C, C], F32, kind="ExternalOutput")
    o1_d = nc.dram_tensor("o1c", [128, NB, RT], F32, kind="ExternalOutput")
    o2_d = nc.dram_tensor("o2c", [128, NB, RT], F32, kind="ExternalOutput")

    send_d = nc.dram_tensor("x_send", [B, CS], F32)
    recv_d = nc.dram_tensor("x_recv", [NCORES, NB, CS], F32)

    with tile.TileContext(nc) as tc:
        with (
            tc.tile_pool(name="consts", bufs=1) as consts,
            tc.tile_pool(name="stats", bufs=1) as stats,
            tc.tile_pool(name="small", bufs=2) as small,
            tc.tile_pool(name="xb", bufs=2) as xb_pool,
            tc.tile_pool(name="e", bufs=4) as e_pool,
            tc.tile_pool(name="att", bufs=4) as att_pool,
            tc.tile_pool(name="psum_x", bufs=1, space="PSUM") as psum_x,
            tc.tile_pool(name="psum_t", bufs=2, space="PSUM") as psum_t,
            tc.tile_pool(name="psum_num", bufs=1, space="PSUM") as psum_num,
        ):
            ident = consts.tile([128, 128], F32)
            make_identity(nc, ident[:])

            xcatT_sb = consts.tile([128, KC, B], F32)
            nc.gpsimd.dma_start(out=xcatT_sb[:], in_=xcatT_d[:])
            x12_sb = consts.tile([128, NB, RT, 2], F32)
            nc.gpsimd.dma_start(out=x12_sb[:], in_=x12_d[:])
            x12r_sb = consts.tile([128, NB, RT, 2], F32R)
            nc.gpsimd.tensor_copy(x12r_sb[:], x12_sb[:])
            b32_sb = consts.tile([B, CS], F32)
            nc.gpsimd.dma_start(out=b32_sb[:], in_=b32_d[:])
            wt3 = consts.tile([128, KC, CS], F32)
            nc.sync.dma_start(
                out=wt3[:], in_=wtsl_d[:].rearrange("(kc p) r -> p kc r", p=128)
            )

            # ---- Phase 0: x[:, my 256 channels] for ALL 32 batches ----
            xps = psum_x.tile([B, CS], F32)
            for kc in range(KC):
                nc.tensor.matmul(
                    xps[:],
                    xcatT_sb[:, kc, :],
                    wt3[:, kc, :],
                    start=(kc == 0),
                    stop=(kc == KC - 1),
                )
            xp_sb = stats.tile([B, CS], F32)
            nc.vector.tensor_add(xp_sb[:], xps[:], b32_sb[:])
            nc.sync.dma_start(out=send_d[:], in_=xp_sb[:])

            # AllToAll chunk g of send = batches [4g, 4g+4); each core
            # receives its own 4 batches x all 8 channel slices.
            nc.gpsimd.collective_compute(
                "AllToAll",
                mybir.AluOpType.bypass,
                replica_groups=[list(range(NCORES))],
                ins=[send_d[:]],
                outs=[recv_d[:]],
            )

            # scol[p, rt%2, b, rt//2] = -x[b, rt*128+p]
            recv_base = recv_d[:]
            scol = stats.tile([128, 2, NB, NCORES], F32)
            for par in range(2):
                for bb in range(NB):
                    nc.gpsimd.dma_start(
                        out=scol[:, par, bb],
                        in_=bass.AP(
                            tensor=recv_base.tensor,
                            offset=recv_base.offset + par * 128 + bb * CS,
                            ap=[[1, 128], [NB * CS, NCORES]],
                        ),
                    )
            nc.vector.tensor_scalar_mul(scol[:], scol[:], -1.0)

            s_col = stats.tile([128, NB, RT], F32)
            rcp_col = stats.tile([128, NB, RT], F32)
            o1_sb = stats.tile([128, NB, RT], F32)
            o2_sb = stats.tile([128, NB, RT], F32)

            # ---- Phase 1: attention tiles + out1/out2 numerators ----
            for b in range(NB):
                # broadcast x[b, :] (all channels) to all 128 partitions
                xb_sb = xb_pool.tile([128, C], F32, tag="xb")
                nc.gpsimd.dma_start(
                    out=xb_sb[:],
                    in_=bass.AP(
                        tensor=recv_base.tensor,
                        offset=recv_base.offset + b * CS,
                        ap=[[0, 128], [NB * CS, NCORES], [1, CS]],
                    ),
                )

                num_p = psum_num.tile([2, C], F32, tag="num")
                for rt in range(RT):
                    # e written as f32r directly by ACT (rate is dtype-
                    # independent) so the PE can stream it at 1 cyc/row;
                    # the normalize reads the same bits as f32.
                    e_t = e_pool.tile([128, C], F32R, tag="e")
                    nc.scalar.activation(
                        e_t[:],
                        xb_sb[:],
                        mybir.ActivationFunctionType.Exp,
                        bias=0.0,
                        scale=scol[:, rt % 2, b, rt // 2 : rt // 2 + 1],
                        accum_out=s_col[:, b, rt : rt + 1],
                    )
                    for jc in range(4):
                        nc.tensor.matmul(
                            num_p[:, jc * 512 : (jc + 1) * 512],
                            x12r_sb[:, b, rt, :],
                            e_t[:, jc * 512 : (jc + 1) * 512],
                            start=(rt == 0),
                            stop=(rt == RT - 1),
                        )
                    nc.vector.reciprocal(
                        rcp_col[:, b, rt : rt + 1], s_col[:, b, rt : rt + 1]
                    )
                    att_t = att_pool.tile([128, C], F32, tag="att")
                    nc.vector.tensor_scalar_mul(
                        att_t[:], e_t[:].bitcast(F32), rcp_col[:, b, rt : rt + 1]
                    )
                    nc.sync.dma_start(
                        out=att_d[b, rt * 128 : (rt + 1) * 128, :], in_=att_t[:]
                    )

                # finals: out_m[f] = num_m[f] / s_f + xm[f], in column layout
                num_sb = small.tile([2, C], F32, tag="numsb")
                nc.vector.tensor_copy(num_sb[:], num_p[:])
                numT_p = psum_t.tile([128, RT, 2], F32, tag="t")
                for rt in range(RT):
                    nc.tensor.transpose(
                        numT_p[:, rt, :],
                        num_sb[:, rt * 128 : (rt + 1) * 128],
                        ident[0:2, 0:2],
                    )
                t1 = small.tile([128, RT], F32, tag="t1")
                nc.vector.tensor_mul(t1[:], numT_p[:, :, 0], rcp_col[:, b, :])
                nc.vector.tensor_add(o1_sb[:, b, :], t1[:], x12_sb[:, b, :, 0])
                t2 = small.tile([128, RT], F32, tag="t2")
                nc.vector.tensor_mul(t2[:], numT_p[:, :, 1], rcp_col[:, b, :])
                nc.vector.tensor_add(o2_sb[:, b, :], t2[:], x12_sb[:, b, :, 1])

            nc.sync.dma_start(out=o1_d[:], in_=o1_sb[:])
            nc.sync.dma_start(out=o2_d[:], in_=o2_sb[:])

    nc.finalize()  # Bacc: runs compile() (wait legalization etc.) + freeze
    return nc


def get_program():
    if "nc" not in _prog_cache:
        _prog_cache["nc"] = _build_program()
    return _prog_cache["nc"]


def make_in_maps(x1, x2, W, b):
    """Host-side shard + relayout. Pure data movement, no math beyond what
    the reference's concatenate does."""
    x1 = np.asarray(x1, np.float32)
    x2 = np.asarray(x2, np.float32)
    W = np.asarray(W, np.float32)
    b = np.asarray(b, np.float32)

    wt = W.T  # (4096, 2048) view
    xcat = np.concatenate([x1, x2], axis=1)  # (32, 4096)
    # xcatT3[p, kc, b] = xcat[b, kc*128 + p], all batches, replicated
    xcatT3 = np.ascontiguousarray(xcat.T.reshape(KC, 128, B).transpose(1, 0, 2))

    in_maps = []
    for c in range(NCORES):
        bs = slice(c * NB, (c + 1) * NB)
        cs = slice(c * CS, (c + 1) * CS)
        wtsl = np.ascontiguousarray(wt[:, cs])  # (4096, 256)
        b32 = np.ascontiguousarray(np.broadcast_to(b[cs], (B, CS)))
        # x12col[p, b, rt, m] = xm[bs][b, rt*128 + p]
        x1c = x1[bs].reshape(NB, RT, 128).transpose(2, 0, 1)
        x2c = x2[bs].reshape(NB, RT, 128).transpose(2, 0, 1)
        x12 = np.ascontiguousarray(np.stack([x1c, x2c], axis=-1))
        in_maps.append(
            {"xcatT3": xcatT3, "wtsl": wtsl, "x12col": x12, "b32": b32}
        )
    return in_maps


def assemble_outputs(results):
    att = np.concatenate([r["att"] for r in results], axis=0)  # (32, 2048, 2048)
    outs = []
    for key in ("o1c", "o2c"):
        # o[p, b, rt] -> out[b, rt*128+p]
        per_core = [
            r[key].transpose(1, 2, 0).reshape(NB, C) for r in results
        ]
        outs.append(np.concatenate(per_core, axis=0).astype(np.float32))
    return outs[0], outs[1], att.astype(np.float32, copy=False)


def kernel(x1, x2, W, b, _trace=False):
    global LAST_RESULTS
    from concourse.bass_utils import run_bass_kernel_spmd

    nc = get_program()
    in_maps = make_in_maps(x1, x2, W, b)
    res = run_bass_kernel_spmd(
        nc, in_maps, core_ids=list(range(NCORES)), trace=_trace
    )
    LAST_RESULTS = res
    return assemble_outputs(res.results)


# revision 24
# speedup vs baseline: 2.1768x; 1.0360x over previous
"""CAM-module (channel attention, H=W=1) Trainium2 kernel.

Reference math (B=32, C=2048):
    x    = concat([x1, x2], 1) @ W.T + b                  # (B, C)
    E    = x[:, :, None] * x[:, None, :]                  # (B, C, C)
    A    = softmax(max_j(E) - E, axis=-1)                 # (B, C, C)
    out1 = A @ x1 + x1 ; out2 = A @ x2 + x2               # (B, C)

Key identities used here:
  * softmax(max_j(E) - E)[i, j] == softmax(-x_i * x_j)[j]  (row constants
    cancel in softmax).  With x ~ N(0,1), |x_i*x_j| < ~30, so exp(-x_i*x_j)
    is safely inside the f32 range and NO max-subtraction is needed:
        A[i, j] = exp(-x_i*x_j) / s_i,   s_i = sum_j exp(-x_i*x_j)
    Each 128-row attention tile is ONE ScalarE activation:
        e = Exp(xb * scale_p),  scale_p = -x_i  (per-partition scalar),
    with s_i accumulated for free via accum_out.
  * exp(-x_i x_j) is symmetric, so the matvec A @ x1 can contract over
    either index:  num1[f] = sum_i x1[i] * e[i, f]  (PE matmul over the
    same tiles), and out1[f] = num1[f] / s_f + x1[f].

Sharding: the conv1x1 x = concat @ W.T is sharded over OUTPUT CHANNELS
(each core reads only W.T[:, c*256:(c+1)*256], 4 MiB) and computed for
all 32 batches; an AllToAll then hands each core the full-channel x rows
of ITS 4 batches (rank-independent access patterns).  The (B,C,C)
attention is batch-sharded: 4 batches per core, 64 MiB of tile writes.
"""

import numpy as np

import concourse.bacc as bacc
import concourse.bass as bass
import concourse.mybir as mybir
import concourse.tile as tile
from concourse.masks import make_identity

F32 = mybir.dt.float32
F32R = mybir.dt.float32r

B, C = 32, 2048
K2 = 2 * C  # 4096 contraction dim of the conv1x1
NCORES = 8
NB = B // NCORES  # batches per core = 4
RT = C // 128  # row tiles per batch = 16
KC = K2 // 128  # k chunks = 32
CS = C // NCORES  # output-channel slice per core = 256

_prog_cache = {}
LAST_RESULTS = None  # BassKernelResults of the most recent run (for test.py)


def _build_program():
    nc = bacc.Bacc(None, target_bir_lowering=False, num_devices=NCORES)

    xcatT_d = nc.dram_tensor("xcatT3", [128, KC, B], F32, kind="ExternalInput")
    wtsl_d = nc.dram_tensor("wtsl", [K2, CS], F32, kind="ExternalInput")
    x12_d = nc.dram_tensor("x12col", [128, NB, RT, 2], F32, kind="ExternalInput")
    b32_d = nc.dram_tensor("b32", [B, CS], F32, kind="ExternalInput")
    att_d = nc.dram_tensor("att", [NB, C, C], F32, kind="ExternalOutput")
    o1_d = nc.dram_tensor("o1c", [128, NB, RT], F32, kind="ExternalOutput")
    o2_d = nc.dram_tensor("o2c", [128, NB, RT], F32, kind="ExternalOutput")

    send_d = nc.dram_tensor("x_send", [B, CS], F32)
    recv_d = nc.dram_tensor("x_recv", [NCORES, NB, CS], F32)

    with tile.TileContext(nc) as tc:
        with (
            tc.tile_pool(name="consts", bufs=1) as consts,
            tc.tile_pool(name="stats", bufs=1) as stats,
            tc.tile_pool(name="small", bufs=2) as small,
            tc.tile_pool(name="xb", bufs=2) as xb_pool,
            tc.tile_pool(name="e", bufs=5) as e_pool,
            tc.tile_pool(name="att", bufs=6) as att_pool,
            tc.tile_pool(name="wt", bufs=2) as wt_pool,
            tc.tile_pool(name="psum_x", bufs=1, space="PSUM") as psum_x,
            tc.tile_pool(name="psum_t", bufs=2, space="PSUM") as psum_t,
            tc.tile_pool(name="psum_num", bufs=1, space="PSUM") as psum_num,
        ):
            ident = consts.tile([128, 128], F32)
            make_identity(nc, ident[:])

            xcatT_sb = consts.tile([128, KC, B], F32)
            nc.gpsimd.dma_start(out=xcatT_sb[:], in_=xcatT_d[:])
            x12_sb = consts.tile([128, NB, RT, 2], F32)
            nc.gpsimd.dma_start(out=x12_sb[:], in_=x12_d[:])
            x12r_sb = consts.tile([128, NB, RT, 2], F32R)
            nc.gpsimd.tensor_copy(x12r_sb[:], x12_sb[:])
            b32_sb = consts.tile([B, CS], F32)
            nc.gpsimd.dma_start(out=b32_sb[:], in_=b32_d[:])
            wtsl_re = wtsl_d[:].rearrange("(kc p) r -> p kc r", p=128)

            # ---- Phase 0: x[:, my 256 channels] for ALL 32 batches ----
            # W.T slice streamed in 4 chunks so the PE starts early.
            KCH = 8  # k-chunks per DMA chunk
            xps = psum_x.tile([B, CS], F32)
            for ch in range(KC // KCH):
                wt3 = wt_pool.tile([128, KCH, CS], F32, tag="wt")
                nc.sync.dma_start(
                    out=wt3[:], in_=wtsl_re[:, ch * KCH : (ch + 1) * KCH, :]
                )
                for k in range(KCH):
                    kc = ch * KCH + k
                    nc.tensor.matmul(
                        xps[:],
                        xcatT_sb[:, kc, :],
                        wt3[:, k, :],
                        start=(kc == 0),
                        stop=(kc == KC - 1),
                    )
            xp_sb = stats.tile([B, CS], F32)
            nc.vector.tensor_add(xp_sb[:], xps[:], b32_sb[:])
            nc.sync.dma_start(out=send_d[:], in_=xp_sb[:])

            # AllToAll chunk g of send = batches [4g, 4g+4); each core
            # receives its own 4 batches x all 8 channel slices.
            nc.gpsimd.collective_compute(
                "AllToAll",
                mybir.AluOpType.bypass,
                replica_groups=[list(range(NCORES))],
                ins=[send_d[:]],
                outs=[recv_d[:]],
            )

            # scol[p, rt%2, b, rt//2] = -x[b, rt*128+p]
            recv_base = recv_d[:]
            scol = stats.tile([128, 2, NB, NCORES], F32)
            for par in range(2):
                for bb in range(NB):
                    nc.gpsimd.dma_start(
                        out=scol[:, par, bb],
                        in_=bass.AP(
                            tensor=recv_base.tensor,
                            offset=recv_base.offset + par * 128 + bb * CS,
                            ap=[[1, 128], [NB * CS, NCORES]],
                        ),
                    )
            nc.vector.tensor_scalar_mul(scol[:], scol[:], -1.0)

            s_col = stats.tile([128, NB, RT], F32)
            rcp_col = stats.tile([128, NB, RT], F32)
            o1_sb = stats.tile([128, NB, RT], F32)
            o2_sb = stats.tile([128, NB, RT], F32)

            # ---- Phase 1: attention tiles + out1/out2 numerators ----
            for b in range(NB):
                # broadcast x[b, :] (all channels) to all 128 partitions
                xb_sb = xb_pool.tile([128, C], F32, tag="xb")
                nc.sync.dma_start(
                    out=xb_sb[:],
                    in_=bass.AP(
                        tensor=recv_base.tensor,
                        offset=recv_base.offset + b * CS,
                        ap=[[0, 128], [NB * CS, NCORES], [1, CS]],
                    ),
                )

                num_p = psum_num.tile([2, C], F32, tag="num")
                for rt in range(RT):
                    # e written as f32r directly by ACT (rate is dtype-
                    # independent) so the PE can stream it at 1 cyc/row;
                    # the normalize reads the same bits as f32.
                    e_t = e_pool.tile([128, C], F32R, tag="e")
                    nc.scalar.activation(
                        e_t[:],
                        xb_sb[:],
                        mybir.ActivationFunctionType.Exp,
                        bias=0.0,
                        scale=scol[:, rt % 2, b, rt // 2 : rt // 2 + 1],
                        accum_out=s_col[:, b, rt : rt + 1],
                    )
                    for jc in range(4):
                        nc.tensor.matmul(
                            num_p[:, jc * 512 : (jc + 1) * 512],
                            x12r_sb[:, b, rt, :],
                            e_t[:, jc * 512 : (jc + 1) * 512],
                            start=(rt == 0),
                            stop=(rt == RT - 1),
                        )
                    nc.vector.reciprocal(
                        rcp_col[:, b, rt : rt + 1], s_col[:, b, rt : rt + 1]
                    )
                    att_t = att_pool.tile([128, C], F32, tag="att")
                    nc.vector.tensor_scalar_mul(
                        att_t[:], e_t[:].bitcast(F32), rcp_col[:, b, rt : rt + 1]
                    )
                    nc.sync.dma_start(
                        out=att_d[b, rt * 128 : (rt + 1) * 128, :], in_=att_t[:]
                    )

                # finals: out_m[f] = num_m[f] / s_f + xm[f], in column layout
                num_sb = small.tile([2, C], F32, tag="numsb")
                nc.vector.tensor_copy(num_sb[:], num_p[:])
                numT_p = psum_t.tile([128, RT, 2], F32, tag="t")
                for rt in range(RT):
                    nc.tensor.transpose(
                        numT_p[:, rt, :],
                        num_sb[:, rt * 128 : (rt + 1) * 128],
                        ident[0:2, 0:2],
                    )
                t1 = small.tile([128, RT], F32, tag="t1")
                nc.vector.tensor_mul(t1[:], numT_p[:, :, 0], rcp_col[:, b, :])
                nc.vector.tensor_add(o1_sb[:, b, :], t1[:], x12_sb[:, b, :, 0])
                t2 = small.tile([128, RT], F32, tag="t2")
                nc.vector.tensor_mul(t2[:], numT_p[:, :, 1], rcp_col[:, b, :])
                nc.vector.tensor_add(o2_sb[:, b, :], t2[:], x12_sb[:, b, :, 1])

            nc.sync.dma_start(out=o1_d[:], in_=o1_sb[:])
            nc.sync.dma_start(out=o2_d[:], in_=o2_sb[:])

    nc.finalize()  # Bacc: runs compile() (wait legalization etc.) + freeze
    return nc


def get_program():
    if "nc" not in _prog_cache:
        _prog_cache["nc"] = _build_program()
    return _prog_cache["nc"]


def make_in_maps(x1, x2, W, b):
    """Host-side shard + relayout. Pure data movement, no math beyond what
    the reference's concatenate does."""
    x1 = np.asarray(x1, np.float32)
    x2 = np.asarray(x2, np.float32)
    W = np.asarray(W, np.float32)
    b = np.asarray(b, np.float32)

    wt = W.T  # (4096, 2048) view
    xcat = np.concatenate([x1, x2], axis=1)  # (32, 4096)
    # xcatT3[p, kc, b] = xcat[b, kc*128 + p], all batches, replicated
    xcatT3 = np.ascontiguousarray(xcat.T.reshape(KC, 128, B).transpose(1, 0, 2))

    in_maps = []
    for c in range(NCORES):
        bs = slice(c * NB, (c + 1) * NB)
        cs = slice(c * CS, (c + 1) * CS)
        wtsl = np.ascontiguousarray(wt[:, cs])  # (4096, 256)
        b32 = np.ascontiguousarray(np.broadcast_to(b[cs], (B, CS)))
        # x12col[p, b, rt, m] = xm[bs][b, rt*128 + p]
        x1c = x1[bs].reshape(NB, RT, 128).transpose(2, 0, 1)
        x2c = x2[bs].reshape(NB, RT, 128).transpose(2, 0, 1)
        x12 = np.ascontiguousarray(np.stack([x1c, x2c], axis=-1))
        in_maps.append(
            {"xcatT3": xcatT3, "wtsl": wtsl, "x12col": x12, "b32": b32}
        )
    return in_maps


def assemble_outputs(results):
    att = np.concatenate([r["att"] for r in results], axis=0)  # (32, 2048, 2048)
    outs = []
    for key in ("o1c", "o2c"):
        # o[p, b, rt] -> out[b, rt*128+p]
        per_core = [
            r[key].transpose(1, 2, 0).reshape(NB, C) for r in results
        ]
        outs.append(np.concatenate(per_core, axis=0).astype(np.float32))
    return outs[0], outs[1], att.astype(np.float32, copy=False)


def kernel(x1, x2, W, b, _trace=False):
    global LAST_RESULTS
    from concourse.bass_utils import run_bass_kernel_spmd

    nc = get_program()
    in_maps = make_in_maps(x1, x2, W, b)
    res = run_bass_kernel_spmd(
        nc, in_maps, core_ids=list(range(NCORES)), trace=_trace
    )
    LAST_RESULTS = res
    return assemble_outputs(res.results)
